# revision 5
# baseline (speedup 1.0000x reference)
"""ChebyConv (K=3) GNN kernel for 8 Trainium2 NeuronCores — v2.

out = x@(W0-W2) + L@c + bias,  c = x@W1 + (L@x)@(2*W2)

v2 vs baseline:
- fp16 pair gather tables (256B rows = 2 nodes); lhsT uses BOTH halves,
  wrong-parity half zeroed via per-half val table; halves summed at quad end.
- Edges grouped by (dest-128-block, idx-range-of-2); masks are [128,128]
  windows built in ONE batched DVE tensor_tensor is_equal per (quad, range).
- Val scaling folded into one batched DVE mult with broadcast vv.
- PSUM [128, 512] per quad zero-filled by a K=1 matmul; all sparse matmuls
  accumulate (start=False).
- c table split into two Shared tensors (one per idx range) with fp16
  AllGather blocks aligned to the range boundary, so phase-2 range-0
  gathers depend only on the first three AGs.
- mask TT emitted before the gathers and val folds split per gather
  call: the strict-FIFO DVE never head-of-line blocks on gather DMAs.
- bias/dense terms via augmented [65, .] matmul (ones row + bias row).
"""

import os
import numpy as np

CHUNK = 128
WIN = 128            # dest window / block size
DQ = 512             # dest rows per quad (PSUM bank free dim)
MAX_CALL_CHUNKS = 32
NC = 8
PRNG = 25600         # pair rows per idx range (int16 < 32768)

LAST_EXEC_NS = None


def _edge_layout(key_of_edge, ngrp, nquad, r, v, h, idx):
    """Shared static slot layout. key = quad*8 + range*4 + blk_in_quad."""
    counts = np.zeros((NC, ngrp), dtype=np.int64)
    orders, keys = [], []
    for ci in range(NC):
        order = np.lexsort((idx[ci], key_of_edge[ci]))
        keys.append(key_of_edge[ci][order])
        orders.append(order)
        counts[ci] = np.bincount(key_of_edge[ci], minlength=ngrp)
    cg = np.maximum(1, -(-counts.max(axis=0) // CHUNK))
    off = np.concatenate(([0], np.cumsum(cg)))
    tot_chunks = int(off[-1])
    tot_slots = tot_chunks * CHUNK
    # block column of every chunk (for PSUM window)
    blkcol = np.zeros(tot_chunks, dtype=np.int64)
    for g in range(ngrp):
        blkcol[off[g]:off[g + 1]] = g % 4
    # calls per (quad, range): split [off[q*8+rg*4], off[q*8+rg*4+4]) into <=32
    calls = []
    for t in range(nquad):
        for rg in range(2):
            c0, c1 = int(off[t * 8 + rg * 4]), int(off[t * 8 + rg * 4 + 4])
            tot = c1 - c0
            nparts = -(-tot // MAX_CALL_CHUNKS) if tot else 0
            k = c0
            for pi in range(nparts):
                n = tot // nparts + (1 if pi < tot % nparts else 0)
                calls.append((t, rg, k, n))
                k += n
    grp_span = [(int(off[t * 8 + rg * 4]), int(off[t * 8 + rg * 4 + 4]))
                for t in range(nquad) for rg in range(2)]

    per_core = []
    for ci in range(NC):
        order = orders[ci]
        cnt = counts[ci]
        rr = np.zeros(tot_slots, dtype=np.float16)
        vv = np.zeros((tot_slots, 2), dtype=np.float16)
        ii = np.zeros(tot_slots, dtype=np.int16)
        within = np.arange(len(order)) - np.repeat(
            np.concatenate(([0], np.cumsum(cnt)))[:-1], cnt)
        slot = off[keys[ci]] * CHUNK + within
        rr[slot] = (r[ci][order] & (WIN - 1)).astype(np.float16)
        vv[slot, h[ci][order]] = v[ci][order].astype(np.float16)
        ii[slot] = idx[ci][order].astype(np.int16)
        rr_t = np.ascontiguousarray(rr.reshape(tot_chunks, CHUNK).T)
        vv_t = np.ascontiguousarray(
            vv.reshape(tot_chunks, CHUNK, 2).transpose(1, 0, 2)
            .reshape(CHUNK, tot_chunks * 2))
        iw = np.ascontiguousarray(ii.reshape(tot_slots // 16, 16).T)
        iw = np.tile(iw, (8, 1))
        per_core.append((rr_t, vv_t, iw))
    nch_max = max(c1 - c0 for c0, c1 in grp_span)
    return per_core, dict(tot_chunks=tot_chunks, tot_slots=tot_slots,
                          calls=calls, grp_span=grp_span, blkcol=blkcol,
                          nch_max=nch_max)


def _host_prep(x, rows, cols, vals, weight, bias):
    N, F = x.shape
    assert F == 64 and N % NC == 0
    shard = N // NC
    nquad = -(-shard // DQ)
    vrows = nquad * DQ
    assert NC * vrows == 2 * 2 * PRNG  # 102400 nodes = 2 ranges of 25600 pairs
    SB = 5 * DQ          # superblock rows for AllGather
    NSB = vrows // SB

    rows = np.asarray(rows).astype(np.int64)
    cols = np.asarray(cols).astype(np.int64)
    vals = np.asarray(vals, dtype=np.float32)
    x = np.asarray(x, dtype=np.float32)
    weight = np.asarray(weight, dtype=np.float32)
    bias = np.asarray(bias, dtype=np.float32)

    bounds = np.searchsorted(rows, np.arange(NC + 1) * shard)
    r_, c_, v_ = [], [], []
    for ci in range(NC):
        e0, e1 = bounds[ci], bounds[ci + 1]
        r_.append(rows[e0:e1] - ci * shard)
        c_.append(cols[e0:e1])
        v_.append(vals[e0:e1])

    key_, h_, idx_ = [], [], []
    key2_, h2_, idx2_ = [], [], []
    for ci in range(NC):
        blk = r_[ci] >> 7
        quad = blk >> 2
        bq = blk & 3
        # spmm1: gather from x pair table
        p = c_[ci] >> 1
        g1 = p // PRNG
        key_.append(quad * 8 + g1 * 4 + bq)
        idx_.append(p - g1 * PRNG)
        h_.append(c_[ci] & 1)
        # spmm2: gather from c_tblA/B; AG blocks aligned to the idx-range
        # boundary (node 51200): rows [2560,2560,1280 | 2560,2560,1280]
        SBnd = np.array([0, 2560, 5120, 6400, 8960, 11520, 12800])
        rrk = c_[ci] // shard
        lr = c_[ci] - rrk * shard
        kb = np.searchsorted(SBnd, lr, side='right') - 1
        Sk = SBnd[kb]
        Lk = SBnd[kb + 1] - Sk
        tix = NC * Sk + rrk * Lk + (lr - Sk)
        p2 = tix >> 1
        g2 = p2 // PRNG
        key2_.append(quad * 8 + g2 * 4 + bq)
        idx2_.append(p2 - g2 * PRNG)
        h2_.append(tix & 1)

    ngrp = nquad * 8
    lay1_cores, lay1 = _edge_layout(key_, ngrp, nquad, r_, v_, h_, idx_)
    lay2_cores, lay2 = _edge_layout(key2_, ngrp, nquad, r_, v_, h2_, idx2_)

    # fp16 pair table of x: [2*PRNG pairs, 128]
    x_pairs = np.zeros((2 * PRNG, 2 * F), dtype=np.float16)
    x_pairs.reshape(-1, F)[:N] = x.astype(np.float16)

    nch_max = max(lay1["nch_max"], lay2["nch_max"])
    iota_rep = np.tile(np.arange(WIN, dtype=np.float16), (128, nch_max))
    w1 = weight[1].astype(np.float16)
    w2s = (2.0 * weight[2]).astype(np.float16)
    # augmented [65, 64]: rows 0..63 = W0 - W2, row 64 = bias
    w0m2b = np.concatenate(
        [(weight[0] - weight[2]), bias[None, :]], axis=0).astype(np.float16)

    core_inputs = []
    for ci in range(NC):
        rr1, vv1, iw1 = lay1_cores[ci]
        rr2, vv2, iw2 = lay2_cores[ci]
        xq = np.zeros((F + 1, vrows), dtype=np.float16)
        lo = ci * shard
        hi = min(lo + vrows, N)
        xq[:F, :hi - lo] = x[lo:hi].astype(np.float16).T
        xq[F, :] = 1.0
        core_inputs.append({
            "xpairs": x_pairs, "xq": xq,
            "rr1": rr1, "vv1": vv1, "i1": iw1,
            "rr2": rr2, "vv2": vv2, "i2": iw2,
            "iota": iota_rep, "w1": w1, "w2s": w2s, "w0m2b": w0m2b,
        })

    meta = dict(N=N, F=F, shard=shard, nquad=nquad, vrows=vrows,
                SB=SB, NSB=NSB, nch_max=nch_max, lay1=lay1, lay2=lay2)
    return core_inputs, meta


def _build_program(meta):
    import concourse.bass as bass  # noqa
    import concourse.mybir as mybir
    import concourse.tile as tile
    from concourse import bacc

    F = meta["F"]
    nquad = meta["nquad"]
    vrows = meta["vrows"]
    SB, NSB = meta["SB"], meta["NSB"]
    nch_max = meta["nch_max"]
    lay1, lay2 = meta["lay1"], meta["lay2"]
    f32, f16, i16 = mybir.dt.float32, mybir.dt.float16, mybir.dt.int16
    AOP = mybir.AluOpType
    ACTF = mybir.ActivationFunctionType

    nc = bacc.Bacc("TRN2", target_bir_lowering=False, debug=False,
                   num_devices=NC, num_swdge_queues=4)
    xpairs = nc.dram_tensor("xpairs", [2 * PRNG, 2 * F], f16,
                            kind="ExternalInput")
    xq = nc.dram_tensor("xq", [F + 1, vrows], f16, kind="ExternalInput")
    edge_dram = {}
    for nm, lay in (("1", lay1), ("2", lay2)):
        edge_dram["rr" + nm] = nc.dram_tensor(
            "rr" + nm, [128, lay["tot_chunks"]], f16, kind="ExternalInput")
        edge_dram["vv" + nm] = nc.dram_tensor(
            "vv" + nm, [128, lay["tot_chunks"] * 2], f16, kind="ExternalInput")
        edge_dram["i" + nm] = nc.dram_tensor(
            "i" + nm, [128, lay["tot_slots"] // 16], i16, kind="ExternalInput")
    iota = nc.dram_tensor("iota", [128, nch_max * WIN], f16,
                          kind="ExternalInput")
    w1 = nc.dram_tensor("w1", [F, F], f16, kind="ExternalInput")
    w2s = nc.dram_tensor("w2s", [F, F], f16, kind="ExternalInput")
    w0m2b = nc.dram_tensor("w0m2b", [F + 1, F], f16, kind="ExternalInput")
    outT = nc.dram_tensor("outT", [F, vrows], f32, kind="ExternalOutput")
    c_shard = nc.dram_tensor("c_shard", [vrows, F], f16)
    c_tblA = nc.dram_tensor("c_tblA", [PRNG, 2 * F], f16, addr_space="Shared")
    c_tblB = nc.dram_tensor("c_tblB", [PRNG, 2 * F], f16, addr_space="Shared")

    gq = [0]

    with tile.TileContext(nc) as tc:
        with tc.tile_pool(name="const", bufs=1) as constp, \
             tc.tile_pool(name="edges", bufs=6) as edgep, \
             tc.tile_pool(name="gbuf", bufs=4) as gp, \
             tc.tile_pool(name="gvbuf", bufs=4) as gvp, \
             tc.tile_pool(name="mask", bufs=4) as mp, \
             tc.tile_pool(name="acc", bufs=3) as accp, \
             tc.tile_pool(name="ps", bufs=3, space="PSUM") as psp, \
             tc.tile_pool(name="psc", bufs=2, space="PSUM") as pscp:

            iota_t = constp.tile([128, nch_max * WIN], f16)
            nc.sync.dma_start(out=iota_t[:], in_=iota[:])
            w1_t = constp.tile([F, F], f16, tag="w1")
            nc.sync.dma_start(out=w1_t[:], in_=w1[:])
            w2s_t = constp.tile([F, F], f16, tag="w2s")
            nc.sync.dma_start(out=w2s_t[:], in_=w2s[:])
            w0m2b_t = constp.tile([F + 1, F], f16, tag="w0m2b")
            nc.sync.dma_start(out=w0m2b_t[:], in_=w0m2b[:])
            xq_t = constp.tile([F + 1, vrows], f16, tag="xq")
            nc.sync.dma_start(out=xq_t[:], in_=xq[:])
            zcol = constp.tile([1, 128], f16, tag="zcol")
            nc.vector.memset(zcol[:], 0.0)

            def spmm_quad(t, tbl, lay, nm, second):
                ps = psp.tile([128, DQ], f32)
                # zero-fill PSUM: K=1 matmul with zero lhsT
                nc.tensor.matmul(out=ps[:], lhsT=zcol[:],
                                 rhs=xq_t[0:1, 0:DQ], start=True, stop=False,
                                 skip_group_check=True)
                last_mm = [None]
                for rg in range(2):
                    gi = t * 2 + rg
                    c0, c1 = lay["grp_span"][gi]
                    nch = c1 - c0
                    if nch == 0:
                        continue
                    rr_t = edgep.tile([128, nch_max], f16, tag="rr")
                    nc.sync.dma_start(out=rr_t[:, :nch],
                                      in_=edge_dram["rr" + nm][:, c0:c1])
                    vv_t = edgep.tile([128, nch_max * 2], f16, tag="vv")
                    nc.sync.dma_start(out=vv_t[:, :nch * 2],
                                      in_=edge_dram["vv" + nm][:, 2 * c0:2 * c1])
                    ix_t = edgep.tile([128, nch_max * 8], i16, tag="ix")
                    nc.sync.dma_start(out=ix_t[:, :nch * 8],
                                      in_=edge_dram["i" + nm][:, c0 * 8:c1 * 8])
                    # batched mask build first: no gather dependency, so the
                    # strict-FIFO DVE can run ahead while gathers drain
                    mask = mp.tile([128, nch_max * WIN], f16, tag="mask")
                    nc.vector.tensor_tensor(
                        out=mask[:, :nch * WIN]
                            .rearrange("p (c w) -> p c w", w=WIN),
                        in0=rr_t[:, :nch, None].broadcast_to([128, nch, WIN]),
                        in1=iota_t[:, :nch * WIN]
                            .rearrange("p (c w) -> p c w", w=WIN),
                        op=AOP.is_equal)
                    g16 = gp.tile([128, nch_max * 128], f16, tag="g")
                    g16v = gvp.tile([128, nch_max * 128], f16, tag="gv")
                    for (tt, rgg, k0, ncall) in lay["calls"]:
                        if tt != t or rgg != rg:
                            continue
                        nidx = ncall * CHUNK
                        rel = k0 - c0
                        # gather via f32-typed view (halves descriptor-gen work)
                        nc.gpsimd.dma_gather(
                            out_ap=g16[:, rel * 128:(rel + ncall) * 128]
                                .bitcast(f32)
                                .rearrange("p (c e) -> p c e", e=64),
                            in_ap=tbl[rg].bitcast(f32),
                            idxs_ap=ix_t[:, rel * 8:rel * 8 + nidx // 16],
                            num_idxs=nidx, num_idxs_reg=nidx, elem_size=64,
                            single_packet=False, queue_num=gq[0] % 4)
                        gq[0] += 1
                        # per-call val fold: starts as soon as THIS call lands
                        nc.vector.tensor_tensor(
                            out=g16v[:, rel * 128:(rel + ncall) * 128]
                                .rearrange("p (k f) -> p k f", f=64),
                            in0=vv_t[:, rel * 2:(rel + ncall) * 2, None]
                                .broadcast_to([128, ncall * 2, 64]),
                            in1=g16[:, rel * 128:(rel + ncall) * 128]
                                .rearrange("p (k f) -> p k f", f=64),
                            op=AOP.mult)
                    for j in range(nch):
                        b = int(lay["blkcol"][c0 + j])
                        mm = nc.tensor.matmul(
                            out=ps[:, b * WIN:(b + 1) * WIN],
                            lhsT=g16v[:, j * 128:(j + 1) * 128],
                            rhs=mask[:, j * WIN:(j + 1) * WIN],
                            start=False, stop=False, skip_group_check=True)
                        last_mm[0] = (c0 + j, b)
                if not second:
                    # c^T stage: t1 halves summed -> t1t16 [64, 512]
                    t1h = accp.tile([F, DQ], f16, tag="t1h")
                    nc.scalar.activation(out=t1h[:], in_=ps[0:F, :],
                                         func=ACTF.Copy)
                    t1t = accp.tile([F, DQ], f16, tag="t1t")
                    nc.vector.tensor_tensor(out=t1t[:], in0=t1h[:],
                                            in1=ps[F:2 * F, :], op=AOP.add)
                    psc = pscp.tile([128, 4 * F], f32)
                    for k in range(4):
                        nc.tensor.matmul(out=psc[:, k * F:(k + 1) * F],
                                         lhsT=t1t[:, k * 128:(k + 1) * 128],
                                         rhs=w2s_t[:], start=True, stop=False)
                        nc.tensor.matmul(out=psc[:, k * F:(k + 1) * F],
                                         lhsT=xq_t[0:F, t * DQ + k * 128:
                                                   t * DQ + (k + 1) * 128],
                                         rhs=w1_t[:], start=False, stop=True)
                    c_sb = accp.tile([128, 4 * F], f16, tag="csb")
                    nc.scalar.activation(out=c_sb[:], in_=psc[:],
                                         func=ACTF.Copy)
                    nc.sync.dma_start(
                        out=c_shard[t * DQ:(t + 1) * DQ, :]
                            .rearrange("(k p) e -> p k e", p=128),
                        in_=c_sb[:].rearrange("p (k e) -> p k e", e=F))
                else:
                    # dense: x@(W0-W2) + bias via augmented [65, .]
                    nc.tensor.matmul(out=ps[0:F, :], lhsT=w0m2b_t[:],
                                     rhs=xq_t[:, t * DQ:(t + 1) * DQ],
                                     start=False, stop=True,
                                     skip_group_check=True)
                    o_h = accp.tile([F, DQ], f32, tag="oh")
                    nc.scalar.activation(out=o_h[:], in_=ps[0:F, :],
                                         func=ACTF.Copy)
                    o_sb = accp.tile([F, DQ], f32, tag="osb")
                    nc.vector.tensor_tensor(out=o_sb[:], in0=o_h[:],
                                            in1=ps[F:2 * F, :], op=AOP.add)
                    nc.sync.dma_start(out=outT[:, t * DQ:(t + 1) * DQ],
                                      in_=o_sb[:])

            SBnd = [0, 2560, 5120, 6400, 8960, 11520, 12800]

            def emit_ag(k):
                i0, i1 = SBnd[k], SBnd[k + 1]
                tbl = c_tblA if k < 3 else c_tblB
                o0 = (NC * i0 - (0 if k < 3 else NC * 6400)) // 2
                nc.gpsimd.collective_compute(
                    "AllGather", mybir.AluOpType.bypass,
                    replica_groups=[list(range(NC))],
                    ins=[c_shard[i0:i1, :]],
                    outs=[tbl[o0:o0 + NC * (i1 - i0) // 2, :]])

            AG_LAG = 1
            pending = [(k, -(-SBnd[k + 1] // DQ) - 1) for k in range(6)]
            xp2 = (xpairs[0:PRNG, :], xpairs[PRNG:2 * PRNG, :])
            for t in range(nquad):
                spmm_quad(t, xp2, lay1, "1", second=False)
                while pending and t >= pending[0][1] + AG_LAG:
                    k, _ = pending.pop(0)
                    emit_ag(k)
            for k, _ in pending:
                emit_ag(k)
            for t in range(nquad):
                spmm_quad(t, (c_tblA[:, :], c_tblB[:, :]), lay2, "2",
                          second=True)

    nc.compile()
    return nc


def kernel(**inputs):
    global LAST_EXEC_NS
    core_inputs, meta = _host_prep(
        inputs["x"], inputs["rows"], inputs["cols"], inputs["vals"],
        inputs["weight"], inputs["bias"])
    nc = _build_program(meta)

    trace = os.environ.get("KERNEL_TRACE", "0") == "1"
    if trace:
        try:
            import sys, types  # noqa
            if "antenv.axon_hooks" not in sys.modules:
                import antenv
                from trn_agent_boot.trn_boot import _ntff_profile_via_ctypes
                mod = types.ModuleType("antenv.axon_hooks")
                hook = _ntff_profile_via_ctypes("/opt/axon/libaxon_pjrt.so")
                mod.get_axon_ntff_profile_hook = lambda: hook
                sys.modules["antenv.axon_hooks"] = mod
                antenv.axon_hooks = mod
        except Exception:
            trace = False

    from concourse.bass_utils import run_bass_kernel_spmd
    res = run_bass_kernel_spmd(nc, core_inputs, list(range(NC)), trace=trace)
    LAST_EXEC_NS = res.exec_time_ns

    N, F, shard = meta["N"], meta["F"], meta["shard"]
    out = np.empty((N, F), dtype=np.float32)
    for ci in range(NC):
        out[ci * shard:(ci + 1) * shard] = res.results[ci]["outT"][:, :shard].T
    return out
